# revision 4
# baseline (speedup 1.0000x reference)
"""Chamfer deviation L2 kernel for Trainium2 (8 NeuronCores, data-parallel over batch).

Contract: kernel(xyz1, xyz2) takes FULL inputs [8, 4096, 3] fp32 and returns the
FULL output [4] fp32 (cd_l2 scalar + 3-vector mean deviation).

Algorithm (per core = one batch b):
  x[n, m] = -d[n, m] = 2*x1.x2 - |x1|^2 - |x2|^2 computed on the PE as a
  K=18 augmented contraction: every fp32 coordinate is split into a bf16
  hi/lo pair so all cross products are exact in the fp32 PSUM accumulator
  (total |error| ~1e-5 vs an fp32 reference) while streaming at the bf16
  rate of 1 cycle/row (plain fp32 matmuls cost 4).
  Two orientations: A tiles [n_part=128, m_free] (weights from xyz1,
  stream xyz2), B the transpose. Per [128, 2048] PSUM half-row the DVE
  Max8 instruction yields the row max (= -dmin) and MaxIndex the argmin
  column; both DMA out. No second production or count matmuls needed.
  Host: dist sums from the maxes; deviation vector sums via a 4096-point
  gather per core/direction; final [4] assembled in float64, cast to fp32.
"""

import sys

sys.path.insert(0, "/opt/trn_rl_repo")

import numpy as np

import concourse.bass as bass
import concourse.bacc as bacc
import concourse.tile as tile
from concourse import mybir
from concourse.bass_utils import run_bass_kernel_spmd

F32 = mybir.dt.float32
BF16 = mybir.dt.bfloat16
FP16 = mybir.dt.float16
U16 = mybir.dt.uint16
AX = mybir.AxisListType
OP = mybir.AluOpType
AF = mybir.ActivationFunctionType

B, N, M = 8, 4096, 4096
P = 128
KA = 18     # augmented contraction rows (bf16 hi/lo split)
CH = 2048   # psum work-tile width (4 banks); 2 halves per 4096 row


def build_nc(n=N, m=M, loop=1, mode="psum"):
    """Build the per-core Bacc program (SPMD: same program on all 8 cores).
    loop > 1 repeats the whole compute body (for wall-clock benchmarking).
    mode: "psum" = Max/MaxIndex read the fp32 PSUM tile directly.
          "fp16" = ScalarE drains PSUM to an fp16 SBUF copy first (frees
                   PSUM sooner and lets the DVE run in a 2x 16-bit mode)."""
    assert n == m and n % CH == 0
    nt = n // P              # weight tiles per orientation (32)
    nh = n // CH             # psum halves per row (2)

    nc = bacc.Bacc("TRN2", target_bir_lowering=False, debug=False)

    d_wA = nc.dram_tensor("wA", [KA, n], BF16, kind="ExternalInput")
    d_sA = nc.dram_tensor("sA", [KA, n], BF16, kind="ExternalInput")
    d_wB = nc.dram_tensor("wB", [KA, n], BF16, kind="ExternalInput")
    d_sB = nc.dram_tensor("sB", [KA, n], BF16, kind="ExternalInput")

    MXDT = F32 if mode == "psum" else FP16
    d_mxA = nc.dram_tensor("mxA", [P, nt * nh * 8], MXDT, kind="ExternalOutput")
    d_ixA = nc.dram_tensor("ixA", [P, nt * nh * 8], U16, kind="ExternalOutput")
    d_mxB = nc.dram_tensor("mxB", [P, nt * nh * 8], MXDT, kind="ExternalOutput")
    d_ixB = nc.dram_tensor("ixB", [P, nt * nh * 8], U16, kind="ExternalOutput")

    with tile.TileContext(nc) as tc:
        from contextlib import ExitStack

        with ExitStack() as ctx:
            cpool = ctx.enter_context(tc.tile_pool(name="const", bufs=1))
            work_ps = ctx.enter_context(
                tc.tile_pool(name="workps", bufs=2, space="PSUM")
            )
            scr_pool = ctx.enter_context(tc.tile_pool(name="scr", bufs=3))

            wA = cpool.tile([KA, n], BF16, tag="wA")
            sA = cpool.tile([KA, n], BF16, tag="sA")
            wB = cpool.tile([KA, n], BF16, tag="wB")
            sB = cpool.tile([KA, n], BF16, tag="sB")
            mxA = cpool.tile([P, nt * nh * 8], MXDT, tag="mxA")
            ixA = cpool.tile([P, nt * nh * 8], U16, tag="ixA")
            mxB = cpool.tile([P, nt * nh * 8], MXDT, tag="mxB")
            ixB = cpool.tile([P, nt * nh * 8], U16, tag="ixB")

            nc.sync.dma_start(wA[:, :], d_wA.ap())
            nc.sync.dma_start(sA[:, :], d_sA.ap())
            nc.sync.dma_start(wB[:, :], d_wB.ap())
            nc.sync.dma_start(sB[:, :], d_sB.ap())

            def direction(w, s, mx_sb, ix_sb):
                for t in range(nt):
                    for h in range(nh):
                        pt = work_ps.tile([P, CH], F32, tag="pt", name="pt")
                        for j in range(CH // 512):
                            nc.tensor.matmul(
                                pt[:, j * 512:(j + 1) * 512],
                                lhsT=w[:, t * P:(t + 1) * P],
                                rhs=s[:, h * CH + j * 512:h * CH + (j + 1) * 512],
                                start=True,
                                stop=True,
                            )
                        o = (t * nh + h) * 8
                        if mode == "psum":
                            nc.vector.max(mx_sb[:, o:o + 8], pt[:, :])
                            nc.vector.max_index(
                                ix_sb[:, o:o + 8], mx_sb[:, o:o + 8], pt[:, :]
                            )
                        else:
                            xf = scr_pool.tile([P, CH], FP16, tag="xf", name="xf")
                            nc.scalar.copy(xf[:, :], pt[:, :])
                            nc.vector.max(mx_sb[:, o:o + 8], xf[:, :])
                            nc.vector.max_index(
                                ix_sb[:, o:o + 8], mx_sb[:, o:o + 8], xf[:, :]
                            )

            def body():
                direction(wA, sA, mxA, ixA)
                direction(wB, sB, mxB, ixB)
                nc.sync.dma_start(d_mxA.ap(), mxA[:, :])
                nc.sync.dma_start(d_ixA.ap(), ixA[:, :])
                nc.sync.dma_start(d_mxB.ap(), mxB[:, :])
                nc.sync.dma_start(d_ixB.ap(), ixB[:, :])

            if loop > 1:
                with tc.For_i(0, loop, 1):
                    body()
            else:
                body()

    nc.compile()
    return nc


def _split_bf16(a):
    """fp32/64 array -> (hi, lo) bf16 pair with hi + lo ~= a (16-bit mantissa)."""
    import ml_dtypes

    hi = a.astype(ml_dtypes.bfloat16)
    lo = (a - hi.astype(np.float64)).astype(ml_dtypes.bfloat16)
    return hi, lo


def _augment(xyz, n):
    """[n,3] fp32 -> (lhs [18,n], rhs [18,n]) bf16 so that for points a (lhs
    side) and b (rhs side):  sum_k lhs_k(a) * rhs_k(b) ~= 2 a.b - |a|^2 - |b|^2
    with every kept product exact in fp32 accumulation.

    Row map (c = x,y,z coords; h/l = bf16 hi/lo parts):
      0-2:  2*ah_c  (x) bh_c      3-5:  2*ah_c (x) bl_c
      6-8:  2*al_c  (x) bh_c      9-11: 2*al_c (x) bl_c
      12-14: -sqh,-sql,-sqll (x) 1
      15-17: -1 (x) sqh,sql,sqll
    """
    import ml_dtypes

    a = xyz.astype(np.float64)
    sq = (a * a).sum(axis=1)
    ch, cl = _split_bf16(a.T)          # [3, n] each
    s0 = sq.astype(ml_dtypes.bfloat16)
    r1 = sq - s0.astype(np.float64)
    s1 = r1.astype(ml_dtypes.bfloat16)
    s2 = (r1 - s1.astype(np.float64)).astype(ml_dtypes.bfloat16)

    one = np.ones(n, ml_dtypes.bfloat16)
    zero = np.zeros(n, ml_dtypes.bfloat16)
    two_ch = (ch.astype(np.float32) * 2.0).astype(ml_dtypes.bfloat16)
    two_cl = (cl.astype(np.float32) * 2.0).astype(ml_dtypes.bfloat16)

    lhs = np.stack(
        [two_ch[0], two_ch[1], two_ch[2],
         two_ch[0], two_ch[1], two_ch[2],
         two_cl[0], two_cl[1], two_cl[2],
         two_cl[0], two_cl[1], two_cl[2],
         -s0, -s1, -s2,
         -one, -one, -one]
    )
    rhs = np.stack(
        [ch[0], ch[1], ch[2],
         cl[0], cl[1], cl[2],
         ch[0], ch[1], ch[2],
         cl[0], cl[1], cl[2],
         one, one, one,
         s0, s1, s2]
    )
    assert lhs.shape == (KA, n) and rhs.shape == (KA, n)
    _ = zero
    return lhs, rhs


def make_inputs(xyz1b, xyz2b, n=N, m=M):
    """Build augmented bf16 operands for one batch."""
    lhs1, rhs1 = _augment(xyz1b, n)
    lhs2, rhs2 = _augment(xyz2b, m)
    return {"wA": lhs1, "sA": rhs2, "wB": lhs2, "sB": rhs1}


def decode_core(out, xyz1b, xyz2b, n=N, m=M, verbose=False):
    """Decode one core's outputs into partial sums (float64)."""
    nt, nh = n // P, n // CH

    def one_dir(mx, ix, other_xyz):
        mx = mx.astype(np.float64).reshape(P, nt, nh, 8)[..., 0]  # [128, nt, nh]
        ix = ix.astype(np.int64).reshape(P, nt, nh, 8)[..., 0]
        hstar = np.argmax(mx, axis=2)                             # [128, nt]
        pi, ti = np.indices((P, nt))
        val = mx[pi, ti, hstar]                                   # row max = -dmin
        gidx = hstar * CH + ix[pi, ti, hstar]                     # global argmin col
        bad = (gidx < 0) | (gidx >= other_xyz.shape[0])
        nbad = int(bad.sum())
        gidx = np.clip(gidx, 0, other_xyz.shape[0] - 1)
        dist_sum = -val.sum()
        V = other_xyz.astype(np.float64)[gidx.ravel()].sum(axis=0)
        return dist_sum, V, nbad

    s1, V1, bad1 = one_dir(out["mxA"], out["ixA"], xyz2b)
    s2, V2, bad2 = one_dir(out["mxB"], out["ixB"], xyz1b)
    if verbose or bad1 or bad2:
        print(f"  dist sums: s1={s1:.3f} s2={s2:.3f} bad_idx={bad1}+{bad2}")
    return s1, s2, V1, V2


_NC_CACHE = {}
LAST_RESULTS = None


def kernel(xyz1, xyz2, trace=False, verbose=False):
    global LAST_RESULTS
    xyz1 = np.asarray(xyz1, dtype=np.float32)
    xyz2 = np.asarray(xyz2, dtype=np.float32)
    b, n, _ = xyz1.shape
    m = xyz2.shape[1]

    key = (n, m)
    if key not in _NC_CACHE:
        _NC_CACHE[key] = build_nc(n, m)
    nc = _NC_CACHE[key]

    in_maps = [make_inputs(xyz1[i], xyz2[i], n, m) for i in range(b)]
    res = run_bass_kernel_spmd(nc, in_maps, core_ids=list(range(b)), trace=trace)
    LAST_RESULTS = res

    S1 = S2 = 0.0
    V1 = np.zeros(3)
    V2 = np.zeros(3)
    for i in range(b):
        s1, s2, v1, v2 = decode_core(
            res.results[i], xyz1[i], xyz2[i], n, m, verbose=verbose
        )
        S1 += s1
        S2 += s2
        V1 += v1
        V2 += v2

    sum1 = xyz1.astype(np.float64).sum(axis=(0, 1))
    sum2 = xyz2.astype(np.float64).sum(axis=(0, 1))
    cd_l2 = S1 / (b * n) + S2 / (b * m)
    cd_dev = (sum1 - V1) / (b * n) + (sum2 - V2) / (b * m)
    return np.concatenate([[cd_l2], cd_dev]).astype(np.float32)
